# revision 18
# baseline (speedup 1.0000x reference)
"""Batched Kalman-gain kernel v3 for Trainium2 (Bass/Tile), 8-core data parallel.

Per batch b: Sigma = F Sp F^T + Q; S = H Sigma H^T + R; KG = Sigma H^T S^-1.
Factored: A = H F; C = Sp A^T; P12 = F C + (H Q)^T; S = H P12 + R;
X = S^-1 (2x2-block Schur, approx-recip); KG = P12 X.

Layout: "planes", prepared HOST-SIDE. 128 SBUF partitions = batch lanes,
each lane holds G batches per chunk. The host pre-arranges every input
tensor into fp16 component-planes [chunk, p, comp, g] (g = batch within
lane, stride-1 innermost), so the device DMAs planes directly and runs
every per-batch product term as an elementwise TT with all operands
stride-1 innermost -> DVE 2x_1P fp16 mode (2 el/cycle/lane). Contraction
sums ride the PE via an fp16 identity stationary accumulating in PSUM
(1 col/cycle); ACT evacuates PSUM->SBUF in whatever plane order the next
stage needs. The SPD 4x4 inverse is a Schur complement on S-planes,
batched over IPAIR chunks, reciprocal_approx_fast.
"""

import os

import numpy as np

P = 128
B = 262144
NCORES = 8
B_CORE = B // NCORES  # 32768

G = int(os.environ.get("KG_G", "16"))
IPAIR = int(os.environ.get("KG_IPAIR", "8"))
ASSIGN = os.environ.get("KG_ASSIGN", "A:v,C:v,FC:v,HQ:v,S:v,KG:v")
INV_ENG = os.environ.get("KG_INV", "v")
NCHUNK = B_CORE // (P * G)

_NC_CACHE = {}


def _build_nc(g=None, ipair=None, assign=None):
    import concourse.bacc as bacc
    import concourse.mybir as mybir
    import concourse.tile as tile
    from concourse.masks import make_identity

    g = G if g is None else g
    ipair = IPAIR if ipair is None else ipair
    assign = ASSIGN if assign is None else assign

    fp32 = mybir.dt.float32
    fp16 = mybir.dt.float16
    MULT = mybir.AluOpType.mult

    eng_of = dict(kv.split(":") for kv in assign.split(","))

    nchunk = B_CORE // (P * g)
    assert nchunk * P * g == B_CORE
    assert nchunk % ipair == 0
    gi = g * ipair

    nc = bacc.Bacc("TRN2", target_bir_lowering=False, debug=False)

    # Host-prepared fp16 plane-layout inputs: [chunk, p, comp..., g]
    F_d = nc.dram_tensor("Fpl", [nchunk, P, 8, 8, g], fp16, kind="ExternalInput").ap()
    H_d = nc.dram_tensor("Hpl", [nchunk, P, 4, 8, g], fp16, kind="ExternalInput").ap()
    Sp_d = nc.dram_tensor(
        "Sppl", [nchunk, P, 8, 8, g], fp16, kind="ExternalInput"
    ).ap()
    Q_d = nc.dram_tensor("Qpl", [nchunk, P, 8, 8, g], fp16, kind="ExternalInput").ap()
    R_d = nc.dram_tensor("Rpl", [nchunk, P, 4, 4, g], fp16, kind="ExternalInput").ap()
    KG_d = nc.dram_tensor("KG", [B_CORE, 8, 4], fp32, kind="ExternalOutput").ap()

    KGv = KG_d.rearrange("(c p g) i m -> c p g i m", p=P, g=g)

    with tile.TileContext(nc) as tc:
        with (
            tc.tile_pool(name="consts", bufs=1) as consts,
            tc.tile_pool(name="planes", bufs=int(os.environ.get("KG_PLB", "4"))) as plp,
            tc.tile_pool(name="prod", bufs=2) as prodp,
            tc.tile_pool(name="prodsk", bufs=int(os.environ.get("KG_SKB", "4"))) as prodskp,
            tc.tile_pool(name="mid", bufs=3) as midp,
            tc.tile_pool(name="p12", bufs=IPAIR + 4) as p12p,
            tc.tile_pool(name="sx", bufs=2) as sxp,
            tc.tile_pool(name="invt", bufs=1) as invp,
            tc.tile_pool(name="out", bufs=4) as outp,
            tc.tile_pool(name="psA", bufs=int(os.environ.get("KG_PSB", "6")), space="PSUM") as psmain,
            tc.tile_pool(name="psB", bufs=int(os.environ.get("KG_PSB2", "2")), space="PSUM") as pssml,
        ):
            ident = consts.tile([P, P], fp32, tag="ident")
            make_identity(nc, ident[:])
            id16_t = consts.tile([P, P], fp16, tag="id16")
            nc.vector.tensor_copy(id16_t[:], ident[:])
            id16 = id16_t[:]

            V = nc.vector
            GP = nc.gpsimd

            def ENG(stage):
                return V if eng_of.get(stage, "v") == "v" else GP

            IV = V if INV_ENG == "v" else GP

            st = [dict() for _ in range(nchunk)]
            inv_st = [dict() for _ in range(nchunk // ipair)]

            def emit_load(c):
                s = st[c]
                Fp = plp.tile([P, 8, 8, g], fp16, tag="Fp", name="Fp")
                Hp = plp.tile([P, 4, 8, g], fp16, tag="Hp", name="Hp")
                Spp = plp.tile([P, 8, 8, g], fp16, tag="Spp", name="Spp")
                Qp = plp.tile([P, 8, 8, g], fp16, tag="Qp", name="Qp")
                R16 = plp.tile([P, 4, 4, g], fp16, tag="R16", name="R16")
                if c == 0:
                    # cold start: stripe F/H across rows so they land on
                    # several DMA queues and arrive ~4x sooner.
                    for i0 in range(0, 8, 2):
                        nc.sync.dma_start(
                            out=Fp[:, i0 : i0 + 2], in_=F_d[c, :, i0 : i0 + 2]
                        )
                    for m0 in range(0, 4, 2):
                        nc.sync.dma_start(
                            out=Hp[:, m0 : m0 + 2], in_=H_d[c, :, m0 : m0 + 2]
                        )
                else:
                    nc.sync.dma_start(out=Fp[:], in_=F_d[c])
                    nc.sync.dma_start(out=Hp[:], in_=H_d[c])
                nc.sync.dma_start(out=Spp[:], in_=Sp_d[c])
                nc.sync.dma_start(out=Qp[:], in_=Q_d[c])
                nc.sync.dma_start(out=R16[:], in_=R_d[c])
                s["Fp"] = Fp[:]
                s["Hp"] = Hp[:]
                s["Spp"] = Spp[:]
                s["Qp"] = Qp[:]
                s["R16"] = R16[:]

            def emit_A(c):
                # A(m,cc) = sum_j Hp(m,j) Fp(j,cc); evac -> Atp planes (cc,m)
                s = st[c]
                prodA = prodp.tile([P, 4, 8, 8, g], fp16, tag="prodA", name="prodA")
                ENG("A").tensor_tensor(
                    prodA[:].rearrange("p m j cc g -> p (m j) cc g"),
                    s["Hp"]
                    .rearrange("p m j g -> p (m j) g")
                    .unsqueeze(2)
                    .broadcast_to([P, 32, 8, g]),
                    s["Fp"]
                    .rearrange("p j cc g -> p (j cc) g")
                    .unsqueeze(1)
                    .broadcast_to([P, 4, 64, g]),
                    op=MULT,
                )
                psA = psmain.tile([P, 4 * 8 * g], fp32, tag="ps", name="psA")
                for j in range(8):
                    nc.tensor.matmul(
                        psA[:],
                        id16,
                        prodA[:, :, j].rearrange("p m cc g -> p m (cc g)"),
                        start=(j == 0),
                        stop=(j == 7),
                    )
                s["Atp"] = midp.tile([P, 8, 4, g], fp16, tag="Atp", name="Atp")
                nc.scalar.copy(
                    s["Atp"][:].rearrange("p cc m g -> p m cc g"),
                    psA[:].rearrange("p (m cc g) -> p m cc g", m=4, g=g),
                )

            def emit_C(c):
                # C(i,m) = sum_cc Spp(i,cc) Atp(cc,m); evac -> Cp planes (i,m)
                s = st[c]
                prodC = prodp.tile([P, 8, 8, 4, g], fp16, tag="prodC", name="prodC")
                ENG("C").tensor_tensor(
                    prodC[:].rearrange("p i cc m g -> p (i cc) m g"),
                    s["Spp"]
                    .rearrange("p i cc g -> p (i cc) g")
                    .unsqueeze(2)
                    .broadcast_to([P, 64, 4, g]),
                    s["Atp"][:]
                    .rearrange("p cc m g -> p (cc m) g")
                    .unsqueeze(1)
                    .broadcast_to([P, 8, 32, g]),
                    op=MULT,
                )
                psC = psmain.tile([P, 8 * 4 * g], fp32, tag="ps", name="psC")
                for cc in range(8):
                    nc.tensor.matmul(
                        psC[:],
                        id16,
                        prodC[:, :, cc].rearrange("p i m g -> p i (m g)"),
                        start=(cc == 0),
                        stop=(cc == 7),
                    )
                s["Cp"] = midp.tile([P, 8, 4, g], fp16, tag="Cp", name="Cp")
                nc.scalar.copy(
                    s["Cp"][:].rearrange("p i m g -> p (i m) g"),
                    psC[:].rearrange("p (im g) -> p im g", g=g),
                )

            def emit_P12(c):
                # P12(i,m) = sum_n Fp(i,n) Cp(n,m) + sum_cc Hp(m,cc) Qp(cc,i)
                s = st[c]
                prodF = prodp.tile([P, 8, 8, 4, g], fp16, tag="prodF", name="prodF")
                ENG("FC").tensor_tensor(
                    prodF[:].rearrange("p i n m g -> p (i n) m g"),
                    s["Fp"]
                    .rearrange("p i n g -> p (i n) g")
                    .unsqueeze(2)
                    .broadcast_to([P, 64, 4, g]),
                    s["Cp"][:]
                    .rearrange("p n m g -> p (n m) g")
                    .unsqueeze(1)
                    .broadcast_to([P, 8, 32, g]),
                    op=MULT,
                )
                prodQ = prodp.tile([P, 4, 8, 8, g], fp16, tag="prodQ", name="prodQ")
                ENG("HQ").tensor_tensor(
                    prodQ[:].rearrange("p m cc i g -> p (m cc) i g"),
                    s["Hp"]
                    .rearrange("p m cc g -> p (m cc) g")
                    .unsqueeze(2)
                    .broadcast_to([P, 32, 8, g]),
                    s["Qp"]
                    .rearrange("p cc i g -> p (cc i) g")
                    .unsqueeze(1)
                    .broadcast_to([P, 4, 64, g]),
                    op=MULT,
                )
                psP = psmain.tile([P, 8 * 4 * g], fp32, tag="ps", name="psP")
                for n in range(8):
                    nc.tensor.matmul(
                        psP[:],
                        id16,
                        prodF[:, :, n].rearrange("p i m g -> p i (m g)"),
                        start=(n == 0),
                        stop=False,
                    )
                psP_mi = psP[:].rearrange("p (i m g) -> p m i g", i=8, m=4, g=g)
                for cc in range(8):
                    nc.tensor.matmul(
                        psP_mi,
                        id16,
                        prodQ[:, :, cc].rearrange("p m i g -> p m (i g)"),
                        start=False,
                        stop=(cc == 7),
                    )
                s["P12p"] = p12p.tile([P, 8, 4, g], fp16, tag="P12p", name="P12p")
                nc.scalar.copy(
                    s["P12p"][:].rearrange("p i m g -> p (i m) g"),
                    psP[:].rearrange("p (im g) -> p im g", g=g),
                )

            def emit_S(c):
                # S(m,n) = sum_i Hp(m,i) P12p(i,n) + R.  S is symmetric and
                # the Schur inverse never reads the (0:2, 2:4) block, so only
                # compute rows m=0..1 x cols n=0..1 and rows m=2..3 x n=0..3
                # (12 of 16 entries).
                s = st[c]
                prodS = prodskp.tile([P, 12, 8, g], fp16, tag="prodS", name="prodS")
                # per-m TTs keep every AP at <=3 free dims.
                # q slots: m0:{0,1}, m1:{2,3}, m2:{4..7}, m3:{8..11}
                for m, q0, nn in ((0, 0, 2), (1, 2, 2), (2, 4, 4), (3, 8, 4)):
                    ENG("S").tensor_tensor(
                        prodS[:, q0 : q0 + nn].rearrange("p n i g -> p i n g"),
                        s["Hp"][:, m]
                        .unsqueeze(2)
                        .broadcast_to([P, 8, nn, g]),
                        s["P12p"][:, :, 0:nn],
                        op=MULT,
                    )
                psS = pssml.tile([P, 12 * g], fp32, tag="psS", name="psS")
                for i in range(8):
                    nc.tensor.matmul(
                        psS[:],
                        id16,
                        prodS[:, :, i].rearrange("p q g -> p q (g)"),
                        start=(i == 0),
                        stop=False,
                    )
                # R additions for the same 12 (m,n) slots
                nc.tensor.matmul(
                    psS[:, 0 : 4 * g],
                    id16,
                    s["R16"][:, 0:2, 0:2],
                    start=False,
                    stop=False,
                )
                nc.tensor.matmul(
                    psS[:, 4 * g :],
                    id16,
                    s["R16"][:, 2:4],
                    start=False,
                    stop=True,
                )
                b = c // ipair
                ph = c % ipair
                ib = inv_st[b]
                if "S2" not in ib:
                    # q slots: 0,1,4,5 (A block), 8,9,12,13 (B), 10,11,14,15 (D)
                    ib["S2"] = sxp.tile([P, 16, gi], fp32, tag="S2", name="S2")
                S2v = ib["S2"][:, :, ph * g : (ph + 1) * g]
                nc.scalar.copy(
                    S2v.rearrange("p (m n) g -> p m n g", m=4)[:, 0:2, 0:2],
                    psS[:, 0 : 4 * g].rearrange("p (m n g) -> p m n g", m=2, g=g),
                )
                nc.scalar.copy(
                    S2v.rearrange("p (m n) g -> p m n g", m=4)[:, 2:4],
                    psS[:, 4 * g :].rearrange("p (m n g) -> p m n g", m=2, g=g),
                )

            def emit_inv(b):
                ib = inv_st[b]
                S2 = ib["S2"]
                X2 = sxp.tile([P, 16, gi], fp16, tag="X2", name="X2")
                ib["X2"] = X2
                W4 = invp.tile([P, 4, gi], fp32, tag="W4", name="W4")
                W4b = invp.tile([P, 4, gi], fp32, tag="W4b", name="W4b")
                u2 = invp.tile([P, 2, gi], fp32, tag="u2", name="u2")
                d0 = invp.tile([P, gi], fp32, tag="d0", name="d0")
                Pi = invp.tile([P, 2, 2, gi], fp32, tag="Pi", name="Pi")
                pw = invp.tile([P, 2, 2, 2, gi], fp32, tag="pw", name="pw")
                W2b = invp.tile([P, 2, 2, gi], fp32, tag="W2b", name="W2b")
                Sc = invp.tile([P, 2, 2, gi], fp32, tag="Sc", name="Sc")
                X22 = invp.tile([P, 2, 2, gi], fp32, tag="X22", name="X22")
                X21n = invp.tile([P, 2, 2, gi], fp32, tag="X21n", name="X21n")
                t4 = invp.tile([P, 2, 2, gi], fp32, tag="t4", name="t4")

                Sq = S2[:]
                Sblk = S2[:].rearrange("p (m n) g -> p m n g", m=4)

                IV.tensor_tensor(u2[:], Sq[:, 0:2], Sq[:, 5:3:-1], op=MULT)
                IV.tensor_sub(d0[:], u2[:, 0], u2[:, 1])
                V.reciprocal_approx_fast(out=W4[:, 0], in_=d0[:])
                nc.scalar.mul(
                    W4[:, 1:3], W4[:, 0].unsqueeze(1).broadcast_to([P, 2, gi]), -1.0
                )
                nc.scalar.copy(W4[:, 3], W4[:, 0])
                IV.tensor_tensor(
                    Pi[:],
                    Sblk[:, 1::-1, 1::-1],
                    W4[:].rearrange("p (q r) g -> p q r g", q=2),
                    op=MULT,
                )
                Bblk = Sblk[:, 2:4, 0:2]
                for si in range(2):
                    IV.tensor_tensor(
                        pw[:, si],
                        Bblk[:, :, si]
                        .unsqueeze(2)
                        .broadcast_to([P, 2, 2, gi]),
                        Pi[:, si]
                        .unsqueeze(1)
                        .broadcast_to([P, 2, 2, gi]),
                        op=MULT,
                    )
                IV.tensor_add(W2b[:], pw[:, 0], pw[:, 1])
                for si in range(2):
                    IV.tensor_tensor(
                        pw[:, si],
                        W2b[:, :, si]
                        .unsqueeze(2)
                        .broadcast_to([P, 2, 2, gi]),
                        Bblk[:, :, si]
                        .unsqueeze(1)
                        .broadcast_to([P, 2, 2, gi]),
                        op=MULT,
                    )
                IV.tensor_add(t4[:], pw[:, 0], pw[:, 1])
                IV.tensor_sub(
                    Sc[:],
                    Sblk[:, 2:4, 2:4],
                    t4[:],
                )
                Scq = Sc[:].rearrange("p q r g -> p (q r) g")
                IV.tensor_tensor(u2[:], Scq[:, 0:2], Scq[:, 3:1:-1], op=MULT)
                IV.tensor_sub(d0[:], u2[:, 0], u2[:, 1])
                V.reciprocal_approx_fast(out=W4b[:, 0], in_=d0[:])
                nc.scalar.mul(
                    W4b[:, 1:3], W4b[:, 0].unsqueeze(1).broadcast_to([P, 2, gi]), -1.0
                )
                nc.scalar.copy(W4b[:, 3], W4b[:, 0])
                ib["_cont"] = (X2, W4b, Pi, pw, W2b, Sc, X22, X21n, t4)

            def emit_inv_b(b):
                ib = inv_st[b]
                (X2, W4b, Pi, pw, W2b, Sc, X22, X21n, t4) = ib["_cont"]
                IV.tensor_tensor(
                    X22[:],
                    Sc[:, 1::-1, 1::-1],
                    W4b[:].rearrange("p (q r) g -> p q r g", q=2),
                    op=MULT,
                )
                for si in range(2):
                    IV.tensor_tensor(
                        pw[:, si],
                        X22[:, :, si]
                        .unsqueeze(2)
                        .broadcast_to([P, 2, 2, gi]),
                        W2b[:, si]
                        .unsqueeze(1)
                        .broadcast_to([P, 2, 2, gi]),
                        op=MULT,
                    )
                IV.tensor_add(X21n[:], pw[:, 0], pw[:, 1])
                Xblk = X2[:].rearrange("p (n m) g -> p n m g", n=4)
                nc.scalar.mul(Xblk[:, 2:4, 0:2], X21n[:], -1.0)
                nc.scalar.mul(
                    Xblk[:, 0:2, 2:4].rearrange("p n mp g -> p mp n g"),
                    X21n[:],
                    -1.0,
                )
                nc.scalar.copy(Xblk[:, 2:4, 2:4], X22[:])
                for si in range(2):
                    IV.tensor_tensor(
                        pw[:, si],
                        W2b[:, si]
                        .unsqueeze(2)
                        .broadcast_to([P, 2, 2, gi]),
                        X21n[:, si]
                        .unsqueeze(1)
                        .broadcast_to([P, 2, 2, gi]),
                        op=MULT,
                    )
                IV.tensor_add(t4[:], pw[:, 0], pw[:, 1])
                IV.tensor_add(
                    Xblk[:, 0:2, 0:2],
                    Pi[:],
                    t4[:],
                )

            def emit_KG(c):
                s = st[c]
                b = c // ipair
                ph = c % ipair
                X2 = inv_st[b]["X2"]
                Xh = X2[:, :, ph * g : (ph + 1) * g]
                prodK = prodskp.tile([P, 8, 4, 4, g], fp16, tag="prodK", name="prodK")
                ENG("KG").tensor_tensor(
                    prodK[:].rearrange("p i n m g -> p (i n) m g"),
                    s["P12p"][:]
                    .rearrange("p i n g -> p (i n) g")
                    .unsqueeze(2)
                    .broadcast_to([P, 32, 4, g]),
                    Xh.unsqueeze(1).broadcast_to([P, 8, 16, g]),
                    op=MULT,
                )
                psK = psmain.tile([P, 8 * 4 * g], fp32, tag="ps", name="psK")
                for n in range(4):
                    nc.tensor.matmul(
                        psK[:],
                        id16,
                        prodK[:, :, n].rearrange("p i m g -> p i (m g)"),
                        start=(n == 0),
                        stop=(n == 3),
                    )
                KGo = outp.tile([P, g, 8, 4], fp32, tag="KGo", name="KGo")
                nc.scalar.copy(
                    KGo[:].rearrange("p g i m -> p (i m) g"),
                    psK[:].rearrange("p (im g) -> p im g", g=g),
                )
                nc.sync.dma_start(out=KGv[c], in_=KGo[:])

            # waves: L(t) | A(t-1) | P12(t-2) | S(t-3) | C(t-1) |
            #        inv(group ending at t-3) | KG two per wave
            kskew = 4 + ipair
            for t in range(nchunk + kskew + 1):
                if t < nchunk:
                    emit_load(t)
                if 0 <= t - 1 < nchunk:
                    emit_A(t - 1)
                if 0 <= t - 2 < nchunk:
                    emit_P12(t - 2)
                if 0 <= t - 3 < nchunk:
                    emit_S(t - 3)
                if 0 <= t - 1 < nchunk:
                    emit_C(t - 1)
                last_b = nchunk // ipair - 1
                if 0 <= t - 3 < nchunk and (t - 3) % ipair == ipair - 1:
                    bb = (t - 3) // ipair
                    emit_inv(bb)
                    if bb == last_b:
                        # drain: nothing to smooth, finish the inverse now
                        emit_inv_b(bb)
                if 0 <= t - 4 < nchunk and (t - 4) % ipair == ipair - 1:
                    if (t - 4) // ipair != last_b:
                        emit_inv_b((t - 4) // ipair)
                # two KGs per wave starting one wave after inv_b
                o = t - kskew
                if o >= 0 and (o + 1) // ipair >= nchunk // ipair - 1:
                    o = o + 1  # last group's inverse finishes a wave earlier
                if o >= 0:
                    b2 = o // ipair
                    oo = o - b2 * ipair
                    if 0 <= oo < ipair // 2:
                        for c2 in (
                            b2 * ipair + 2 * oo,
                            b2 * ipair + 2 * oo + 1,
                        ):
                            if 0 <= c2 < nchunk:
                                emit_KG(c2)

    nc.compile()
    return nc


def _get_nc():
    if "nc" not in _NC_CACHE:
        _NC_CACHE["nc"] = _build_nc()
    return _NC_CACHE["nc"]


def _planes(x, comp_shape):
    # [B_CORE, *comp] f32 -> [nchunk, P, *comp, G] fp16 contiguous
    x = x.reshape(NCHUNK, P, G, *comp_shape).astype(np.float16)
    nd = x.ndim
    x = np.transpose(x, (0, 1, *range(3, nd), 2))
    return np.ascontiguousarray(x)


def kernel(F, H, Sigma_previous, Q, R):
    from concourse.bass_utils import run_bass_kernel_spmd

    nc = _get_nc()
    in_maps = [host_shard(F, H, Sigma_previous, Q, R, ci) for ci in range(NCORES)]
    res = run_bass_kernel_spmd(nc, in_maps, core_ids=list(range(NCORES)))
    return np.concatenate([r["KG"] for r in res.results], axis=0)


def host_shard(F, H, Sigma_previous, Q, R, ci):
    sl = slice(ci * B_CORE, (ci + 1) * B_CORE)
    return {
        "Fpl": _planes(np.asarray(F[sl], dtype=np.float32), (8, 8)),
        "Hpl": _planes(np.asarray(H[sl], dtype=np.float32), (4, 8)),
        "Sppl": _planes(np.asarray(Sigma_previous[sl], dtype=np.float32), (8, 8)),
        "Qpl": _planes(np.asarray(Q[sl], dtype=np.float32), (8, 8)),
        "Rpl": _planes(np.asarray(R[sl], dtype=np.float32), (4, 4)),
    }


# revision 19
# speedup vs baseline: 1.1882x; 1.1882x over previous
"""Batched Kalman-gain kernel v3 for Trainium2 (Bass/Tile), 8-core data parallel.

Per batch b: Sigma = F Sp F^T + Q; S = H Sigma H^T + R; KG = Sigma H^T S^-1.
Factored: A = H F; C = Sp A^T; P12 = F C + (H Q)^T; S = H P12 + R
(S symmetric: only the 12 entries the Schur inverse reads are built);
X = S^-1 (2x2-block Schur, approx-recip); KG = P12 X.

Layout: fp16 component-planes [chunk, p, comp, g], prepared HOST-side in
kernel() as part of sharding (g = batch-within-lane, stride-1 innermost;
128 SBUF partitions = batch lanes). The device DMAs planes directly, so
every per-batch product term is an elementwise TT with all operands
stride-1 innermost -> DVE 2x_1P fp16 mode (2 el/cycle/lane, the DVE cap
on TRN2). Contraction sums ride the PE via an fp16 identity stationary
accumulating in PSUM (1 col/cycle); ACT evacuates PSUM->SBUF into the
plane order the next stage needs and also carries the inverse's
scale/copy ops. The SPD 4x4 inverse is a Schur complement on S-planes
batched over IPAIR chunks (fp32, reciprocal_approx_fast on DVE).

Measured on 8xTRN2: DVE-bound (~84% busy); products are the floor.
GPSIMD offload of products or the inverse regresses (chip-wide activity
throttle + 4x worse el/cycle) -- keep KG_ASSIGN/KG_INV at defaults.
"""

import os

import numpy as np

P = 128
B = 262144
NCORES = 8
B_CORE = B // NCORES  # 32768

G = int(os.environ.get("KG_G", "16"))
IPAIR = int(os.environ.get("KG_IPAIR", "8"))
ASSIGN = os.environ.get("KG_ASSIGN", "A:v,C:v,FC:v,HQ:v,S:v,KG:v")
INV_ENG = os.environ.get("KG_INV", "v")
NCHUNK = B_CORE // (P * G)

_NC_CACHE = {}


def _build_nc(g=None, ipair=None, assign=None):
    import concourse.bacc as bacc
    import concourse.mybir as mybir
    import concourse.tile as tile
    from concourse.masks import make_identity

    g = G if g is None else g
    ipair = IPAIR if ipair is None else ipair
    assign = ASSIGN if assign is None else assign

    fp32 = mybir.dt.float32
    fp16 = mybir.dt.float16
    MULT = mybir.AluOpType.mult

    eng_of = dict(kv.split(":") for kv in assign.split(","))

    nchunk = B_CORE // (P * g)
    assert nchunk * P * g == B_CORE
    assert nchunk % ipair == 0
    gi = g * ipair

    nc = bacc.Bacc("TRN2", target_bir_lowering=False, debug=False)

    # Host-prepared fp16 plane-layout inputs: [chunk, p, comp..., g]
    F_d = nc.dram_tensor("Fpl", [nchunk, P, 8, 8, g], fp16, kind="ExternalInput").ap()
    H_d = nc.dram_tensor("Hpl", [nchunk, P, 4, 8, g], fp16, kind="ExternalInput").ap()
    Sp_d = nc.dram_tensor(
        "Sppl", [nchunk, P, 8, 8, g], fp16, kind="ExternalInput"
    ).ap()
    Q_d = nc.dram_tensor("Qpl", [nchunk, P, 8, 8, g], fp16, kind="ExternalInput").ap()
    R_d = nc.dram_tensor("Rpl", [nchunk, P, 4, 4, g], fp16, kind="ExternalInput").ap()
    KG_d = nc.dram_tensor("KG", [B_CORE, 8, 4], fp32, kind="ExternalOutput").ap()

    KGv = KG_d.rearrange("(c p g) i m -> c p g i m", p=P, g=g)

    with tile.TileContext(nc) as tc:
        with (
            tc.tile_pool(name="consts", bufs=1) as consts,
            tc.tile_pool(name="planes", bufs=int(os.environ.get("KG_PLB", "4"))) as plp,
            tc.tile_pool(name="prod", bufs=2) as prodp,
            tc.tile_pool(name="prodsk", bufs=int(os.environ.get("KG_SKB", "4"))) as prodskp,
            tc.tile_pool(name="mid", bufs=3) as midp,
            tc.tile_pool(name="p12", bufs=IPAIR + 4) as p12p,
            tc.tile_pool(name="sx", bufs=2) as sxp,
            tc.tile_pool(name="invt", bufs=1) as invp,
            tc.tile_pool(name="out", bufs=4) as outp,
            tc.tile_pool(name="psA", bufs=int(os.environ.get("KG_PSB", "6")), space="PSUM") as psmain,
            tc.tile_pool(name="psB", bufs=int(os.environ.get("KG_PSB2", "2")), space="PSUM") as pssml,
        ):
            ident = consts.tile([P, P], fp32, tag="ident")
            make_identity(nc, ident[:])
            id16_t = consts.tile([P, P], fp16, tag="id16")
            nc.vector.tensor_copy(id16_t[:], ident[:])
            id16 = id16_t[:]

            V = nc.vector
            GP = nc.gpsimd

            def ENG(stage):
                return V if eng_of.get(stage, "v") == "v" else GP

            IV = V if INV_ENG == "v" else GP

            st = [dict() for _ in range(nchunk)]
            inv_st = [dict() for _ in range(nchunk // ipair)]

            def emit_load(c):
                s = st[c]
                Fp = plp.tile([P, 8, 8, g], fp16, tag="Fp", name="Fp")
                Hp = plp.tile([P, 4, 8, g], fp16, tag="Hp", name="Hp")
                Spp = plp.tile([P, 8, 8, g], fp16, tag="Spp", name="Spp")
                Qp = plp.tile([P, 8, 8, g], fp16, tag="Qp", name="Qp")
                R16 = plp.tile([P, 4, 4, g], fp16, tag="R16", name="R16")
                if c == 0:
                    # cold start: stripe F/H across rows so they land on
                    # several DMA queues and arrive ~4x sooner.
                    for i0 in range(0, 8, 2):
                        nc.sync.dma_start(
                            out=Fp[:, i0 : i0 + 2], in_=F_d[c, :, i0 : i0 + 2]
                        )
                    for m0 in range(0, 4, 2):
                        nc.sync.dma_start(
                            out=Hp[:, m0 : m0 + 2], in_=H_d[c, :, m0 : m0 + 2]
                        )
                else:
                    nc.sync.dma_start(out=Fp[:], in_=F_d[c])
                    nc.sync.dma_start(out=Hp[:], in_=H_d[c])
                nc.sync.dma_start(out=Spp[:], in_=Sp_d[c])
                nc.sync.dma_start(out=Qp[:], in_=Q_d[c])
                nc.sync.dma_start(out=R16[:], in_=R_d[c])
                s["Fp"] = Fp[:]
                s["Hp"] = Hp[:]
                s["Spp"] = Spp[:]
                s["Qp"] = Qp[:]
                s["R16"] = R16[:]

            def emit_A(c):
                # A(m,cc) = sum_j Hp(m,j) Fp(j,cc); evac -> Atp planes (cc,m)
                s = st[c]
                prodA = prodp.tile([P, 4, 8, 8, g], fp16, tag="prodA", name="prodA")
                ENG("A").tensor_tensor(
                    prodA[:].rearrange("p m j cc g -> p (m j) cc g"),
                    s["Hp"]
                    .rearrange("p m j g -> p (m j) g")
                    .unsqueeze(2)
                    .broadcast_to([P, 32, 8, g]),
                    s["Fp"]
                    .rearrange("p j cc g -> p (j cc) g")
                    .unsqueeze(1)
                    .broadcast_to([P, 4, 64, g]),
                    op=MULT,
                )
                psA = psmain.tile([P, 4 * 8 * g], fp32, tag="ps", name="psA")
                for j in range(8):
                    nc.tensor.matmul(
                        psA[:],
                        id16,
                        prodA[:, :, j].rearrange("p m cc g -> p m (cc g)"),
                        start=(j == 0),
                        stop=(j == 7),
                    )
                s["Atp"] = midp.tile([P, 8, 4, g], fp16, tag="Atp", name="Atp")
                nc.scalar.copy(
                    s["Atp"][:].rearrange("p cc m g -> p m cc g"),
                    psA[:].rearrange("p (m cc g) -> p m cc g", m=4, g=g),
                )

            def emit_C(c):
                # C(i,m) = sum_cc Spp(i,cc) Atp(cc,m); evac -> Cp planes (i,m)
                s = st[c]
                prodC = prodp.tile([P, 8, 8, 4, g], fp16, tag="prodC", name="prodC")
                ENG("C").tensor_tensor(
                    prodC[:].rearrange("p i cc m g -> p (i cc) m g"),
                    s["Spp"]
                    .rearrange("p i cc g -> p (i cc) g")
                    .unsqueeze(2)
                    .broadcast_to([P, 64, 4, g]),
                    s["Atp"][:]
                    .rearrange("p cc m g -> p (cc m) g")
                    .unsqueeze(1)
                    .broadcast_to([P, 8, 32, g]),
                    op=MULT,
                )
                psC = psmain.tile([P, 8 * 4 * g], fp32, tag="ps", name="psC")
                for cc in range(8):
                    nc.tensor.matmul(
                        psC[:],
                        id16,
                        prodC[:, :, cc].rearrange("p i m g -> p i (m g)"),
                        start=(cc == 0),
                        stop=(cc == 7),
                    )
                s["Cp"] = midp.tile([P, 8, 4, g], fp16, tag="Cp", name="Cp")
                nc.scalar.copy(
                    s["Cp"][:].rearrange("p i m g -> p (i m) g"),
                    psC[:].rearrange("p (im g) -> p im g", g=g),
                )

            def emit_P12(c):
                # P12(i,m) = sum_n Fp(i,n) Cp(n,m) + sum_cc Hp(m,cc) Qp(cc,i)
                s = st[c]
                prodF = prodp.tile([P, 8, 8, 4, g], fp16, tag="prodF", name="prodF")
                ENG("FC").tensor_tensor(
                    prodF[:].rearrange("p i n m g -> p (i n) m g"),
                    s["Fp"]
                    .rearrange("p i n g -> p (i n) g")
                    .unsqueeze(2)
                    .broadcast_to([P, 64, 4, g]),
                    s["Cp"][:]
                    .rearrange("p n m g -> p (n m) g")
                    .unsqueeze(1)
                    .broadcast_to([P, 8, 32, g]),
                    op=MULT,
                )
                prodQ = prodp.tile([P, 4, 8, 8, g], fp16, tag="prodQ", name="prodQ")
                ENG("HQ").tensor_tensor(
                    prodQ[:].rearrange("p m cc i g -> p (m cc) i g"),
                    s["Hp"]
                    .rearrange("p m cc g -> p (m cc) g")
                    .unsqueeze(2)
                    .broadcast_to([P, 32, 8, g]),
                    s["Qp"]
                    .rearrange("p cc i g -> p (cc i) g")
                    .unsqueeze(1)
                    .broadcast_to([P, 4, 64, g]),
                    op=MULT,
                )
                psP = psmain.tile([P, 8 * 4 * g], fp32, tag="ps", name="psP")
                for n in range(8):
                    nc.tensor.matmul(
                        psP[:],
                        id16,
                        prodF[:, :, n].rearrange("p i m g -> p i (m g)"),
                        start=(n == 0),
                        stop=False,
                    )
                psP_mi = psP[:].rearrange("p (i m g) -> p m i g", i=8, m=4, g=g)
                for cc in range(8):
                    nc.tensor.matmul(
                        psP_mi,
                        id16,
                        prodQ[:, :, cc].rearrange("p m i g -> p m (i g)"),
                        start=False,
                        stop=(cc == 7),
                    )
                s["P12p"] = p12p.tile([P, 8, 4, g], fp16, tag="P12p", name="P12p")
                nc.scalar.copy(
                    s["P12p"][:].rearrange("p i m g -> p (i m) g"),
                    psP[:].rearrange("p (im g) -> p im g", g=g),
                )

            def emit_S(c):
                # S(m,n) = sum_i Hp(m,i) P12p(i,n) + R.  S is symmetric and
                # the Schur inverse never reads the (0:2, 2:4) block, so only
                # compute rows m=0..1 x cols n=0..1 and rows m=2..3 x n=0..3
                # (12 of 16 entries).
                s = st[c]
                prodS = prodskp.tile([P, 12, 8, g], fp16, tag="prodS", name="prodS")
                # per-m TTs keep every AP at <=3 free dims.
                # q slots: m0:{0,1}, m1:{2,3}, m2:{4..7}, m3:{8..11}
                for m, q0, nn in ((0, 0, 2), (1, 2, 2), (2, 4, 4), (3, 8, 4)):
                    ENG("S").tensor_tensor(
                        prodS[:, q0 : q0 + nn].rearrange("p n i g -> p i n g"),
                        s["Hp"][:, m]
                        .unsqueeze(2)
                        .broadcast_to([P, 8, nn, g]),
                        s["P12p"][:, :, 0:nn],
                        op=MULT,
                    )
                psS = pssml.tile([P, 12 * g], fp32, tag="psS", name="psS")
                for i in range(8):
                    nc.tensor.matmul(
                        psS[:],
                        id16,
                        prodS[:, :, i].rearrange("p q g -> p q (g)"),
                        start=(i == 0),
                        stop=False,
                    )
                # R additions for the same 12 (m,n) slots
                nc.tensor.matmul(
                    psS[:, 0 : 4 * g],
                    id16,
                    s["R16"][:, 0:2, 0:2],
                    start=False,
                    stop=False,
                )
                nc.tensor.matmul(
                    psS[:, 4 * g :],
                    id16,
                    s["R16"][:, 2:4],
                    start=False,
                    stop=True,
                )
                b = c // ipair
                ph = c % ipair
                ib = inv_st[b]
                if "S2" not in ib:
                    # q slots: 0,1,4,5 (A block), 8,9,12,13 (B), 10,11,14,15 (D)
                    ib["S2"] = sxp.tile([P, 16, gi], fp32, tag="S2", name="S2")
                S2v = ib["S2"][:, :, ph * g : (ph + 1) * g]
                nc.scalar.copy(
                    S2v.rearrange("p (m n) g -> p m n g", m=4)[:, 0:2, 0:2],
                    psS[:, 0 : 4 * g].rearrange("p (m n g) -> p m n g", m=2, g=g),
                )
                nc.scalar.copy(
                    S2v.rearrange("p (m n) g -> p m n g", m=4)[:, 2:4],
                    psS[:, 4 * g :].rearrange("p (m n g) -> p m n g", m=2, g=g),
                )

            def emit_inv(b):
                ib = inv_st[b]
                S2 = ib["S2"]
                X2 = sxp.tile([P, 16, gi], fp16, tag="X2", name="X2")
                ib["X2"] = X2
                W4 = invp.tile([P, 4, gi], fp32, tag="W4", name="W4")
                W4b = invp.tile([P, 4, gi], fp32, tag="W4b", name="W4b")
                u2 = invp.tile([P, 2, gi], fp32, tag="u2", name="u2")
                d0 = invp.tile([P, gi], fp32, tag="d0", name="d0")
                Pi = invp.tile([P, 2, 2, gi], fp32, tag="Pi", name="Pi")
                pw = invp.tile([P, 2, 2, 2, gi], fp32, tag="pw", name="pw")
                W2b = invp.tile([P, 2, 2, gi], fp32, tag="W2b", name="W2b")
                Sc = invp.tile([P, 2, 2, gi], fp32, tag="Sc", name="Sc")
                X22 = invp.tile([P, 2, 2, gi], fp32, tag="X22", name="X22")
                X21n = invp.tile([P, 2, 2, gi], fp32, tag="X21n", name="X21n")
                t4 = invp.tile([P, 2, 2, gi], fp32, tag="t4", name="t4")

                Sq = S2[:]
                Sblk = S2[:].rearrange("p (m n) g -> p m n g", m=4)

                IV.tensor_tensor(u2[:], Sq[:, 0:2], Sq[:, 5:3:-1], op=MULT)
                IV.tensor_sub(d0[:], u2[:, 0], u2[:, 1])
                V.reciprocal_approx_fast(out=W4[:, 0], in_=d0[:])
                nc.scalar.mul(
                    W4[:, 1:3], W4[:, 0].unsqueeze(1).broadcast_to([P, 2, gi]), -1.0
                )
                nc.scalar.copy(W4[:, 3], W4[:, 0])
                IV.tensor_tensor(
                    Pi[:],
                    Sblk[:, 1::-1, 1::-1],
                    W4[:].rearrange("p (q r) g -> p q r g", q=2),
                    op=MULT,
                )
                Bblk = Sblk[:, 2:4, 0:2]
                for si in range(2):
                    IV.tensor_tensor(
                        pw[:, si],
                        Bblk[:, :, si]
                        .unsqueeze(2)
                        .broadcast_to([P, 2, 2, gi]),
                        Pi[:, si]
                        .unsqueeze(1)
                        .broadcast_to([P, 2, 2, gi]),
                        op=MULT,
                    )
                IV.tensor_add(W2b[:], pw[:, 0], pw[:, 1])
                for si in range(2):
                    IV.tensor_tensor(
                        pw[:, si],
                        W2b[:, :, si]
                        .unsqueeze(2)
                        .broadcast_to([P, 2, 2, gi]),
                        Bblk[:, :, si]
                        .unsqueeze(1)
                        .broadcast_to([P, 2, 2, gi]),
                        op=MULT,
                    )
                IV.tensor_add(t4[:], pw[:, 0], pw[:, 1])
                IV.tensor_sub(
                    Sc[:],
                    Sblk[:, 2:4, 2:4],
                    t4[:],
                )
                Scq = Sc[:].rearrange("p q r g -> p (q r) g")
                IV.tensor_tensor(u2[:], Scq[:, 0:2], Scq[:, 3:1:-1], op=MULT)
                IV.tensor_sub(d0[:], u2[:, 0], u2[:, 1])
                V.reciprocal_approx_fast(out=W4b[:, 0], in_=d0[:])
                nc.scalar.mul(
                    W4b[:, 1:3], W4b[:, 0].unsqueeze(1).broadcast_to([P, 2, gi]), -1.0
                )
                nc.scalar.copy(W4b[:, 3], W4b[:, 0])
                ib["_cont"] = (X2, W4b, Pi, pw, W2b, Sc, X22, X21n, t4)

            def emit_inv_b(b):
                ib = inv_st[b]
                (X2, W4b, Pi, pw, W2b, Sc, X22, X21n, t4) = ib["_cont"]
                IV.tensor_tensor(
                    X22[:],
                    Sc[:, 1::-1, 1::-1],
                    W4b[:].rearrange("p (q r) g -> p q r g", q=2),
                    op=MULT,
                )
                for si in range(2):
                    IV.tensor_tensor(
                        pw[:, si],
                        X22[:, :, si]
                        .unsqueeze(2)
                        .broadcast_to([P, 2, 2, gi]),
                        W2b[:, si]
                        .unsqueeze(1)
                        .broadcast_to([P, 2, 2, gi]),
                        op=MULT,
                    )
                IV.tensor_add(X21n[:], pw[:, 0], pw[:, 1])
                Xblk = X2[:].rearrange("p (n m) g -> p n m g", n=4)
                nc.scalar.mul(Xblk[:, 2:4, 0:2], X21n[:], -1.0)
                nc.scalar.mul(
                    Xblk[:, 0:2, 2:4].rearrange("p n mp g -> p mp n g"),
                    X21n[:],
                    -1.0,
                )
                nc.scalar.copy(Xblk[:, 2:4, 2:4], X22[:])
                for si in range(2):
                    IV.tensor_tensor(
                        pw[:, si],
                        W2b[:, si]
                        .unsqueeze(2)
                        .broadcast_to([P, 2, 2, gi]),
                        X21n[:, si]
                        .unsqueeze(1)
                        .broadcast_to([P, 2, 2, gi]),
                        op=MULT,
                    )
                IV.tensor_add(t4[:], pw[:, 0], pw[:, 1])
                IV.tensor_add(
                    Xblk[:, 0:2, 0:2],
                    Pi[:],
                    t4[:],
                )

            def emit_KG(c):
                s = st[c]
                b = c // ipair
                ph = c % ipair
                X2 = inv_st[b]["X2"]
                Xh = X2[:, :, ph * g : (ph + 1) * g]
                prodK = prodskp.tile([P, 8, 4, 4, g], fp16, tag="prodK", name="prodK")
                ENG("KG").tensor_tensor(
                    prodK[:].rearrange("p i n m g -> p (i n) m g"),
                    s["P12p"][:]
                    .rearrange("p i n g -> p (i n) g")
                    .unsqueeze(2)
                    .broadcast_to([P, 32, 4, g]),
                    Xh.unsqueeze(1).broadcast_to([P, 8, 16, g]),
                    op=MULT,
                )
                psK = psmain.tile([P, 8 * 4 * g], fp32, tag="ps", name="psK")
                for n in range(4):
                    nc.tensor.matmul(
                        psK[:],
                        id16,
                        prodK[:, :, n].rearrange("p i m g -> p i (m g)"),
                        start=(n == 0),
                        stop=(n == 3),
                    )
                KGo = outp.tile([P, g, 8, 4], fp32, tag="KGo", name="KGo")
                nc.scalar.copy(
                    KGo[:].rearrange("p g i m -> p (i m) g"),
                    psK[:].rearrange("p (im g) -> p im g", g=g),
                )
                nc.sync.dma_start(out=KGv[c], in_=KGo[:])

            # waves: L(t) | A(t-1) | P12(t-2) | S(t-3) | C(t-1) |
            #        inv(group ending at t-3) | KG two per wave
            kskew = 4 + ipair
            for t in range(nchunk + kskew + 1):
                if t < nchunk:
                    emit_load(t)
                if 0 <= t - 1 < nchunk:
                    emit_A(t - 1)
                if 0 <= t - 2 < nchunk:
                    emit_P12(t - 2)
                if 0 <= t - 3 < nchunk:
                    emit_S(t - 3)
                if 0 <= t - 1 < nchunk:
                    emit_C(t - 1)
                last_b = nchunk // ipair - 1
                if 0 <= t - 3 < nchunk and (t - 3) % ipair == ipair - 1:
                    bb = (t - 3) // ipair
                    emit_inv(bb)
                    if bb == last_b:
                        # drain: nothing to smooth, finish the inverse now
                        emit_inv_b(bb)
                if 0 <= t - 4 < nchunk and (t - 4) % ipair == ipair - 1:
                    if (t - 4) // ipair != last_b:
                        emit_inv_b((t - 4) // ipair)
                # two KGs per wave starting one wave after inv_b
                o = t - kskew
                if o >= 0 and (o + 1) // ipair >= nchunk // ipair - 1:
                    o = o + 1  # last group's inverse finishes a wave earlier
                if o >= 0:
                    b2 = o // ipair
                    oo = o - b2 * ipair
                    if 0 <= oo < ipair // 2:
                        for c2 in (
                            b2 * ipair + 2 * oo,
                            b2 * ipair + 2 * oo + 1,
                        ):
                            if 0 <= c2 < nchunk:
                                emit_KG(c2)

    nc.compile()
    return nc


def _get_nc():
    if "nc" not in _NC_CACHE:
        _NC_CACHE["nc"] = _build_nc()
    return _NC_CACHE["nc"]


def _planes(x, comp_shape):
    # [B_CORE, *comp] f32 -> [nchunk, P, *comp, G] fp16 contiguous
    x = x.reshape(NCHUNK, P, G, *comp_shape).astype(np.float16)
    nd = x.ndim
    x = np.transpose(x, (0, 1, *range(3, nd), 2))
    return np.ascontiguousarray(x)


def kernel(F, H, Sigma_previous, Q, R):
    from concourse.bass_utils import run_bass_kernel_spmd

    nc = _get_nc()
    in_maps = [host_shard(F, H, Sigma_previous, Q, R, ci) for ci in range(NCORES)]
    res = run_bass_kernel_spmd(nc, in_maps, core_ids=list(range(NCORES)))
    return np.concatenate([r["KG"] for r in res.results], axis=0)


def host_shard(F, H, Sigma_previous, Q, R, ci):
    sl = slice(ci * B_CORE, (ci + 1) * B_CORE)
    return {
        "Fpl": _planes(np.asarray(F[sl], dtype=np.float32), (8, 8)),
        "Hpl": _planes(np.asarray(H[sl], dtype=np.float32), (4, 8)),
        "Sppl": _planes(np.asarray(Sigma_previous[sl], dtype=np.float32), (8, 8)),
        "Qpl": _planes(np.asarray(Q[sl], dtype=np.float32), (8, 8)),
        "Rpl": _planes(np.asarray(R[sl], dtype=np.float32), (4, 4)),
    }


# revision 20
# speedup vs baseline: 1.2012x; 1.0109x over previous
"""Batched Kalman-gain kernel v3 for Trainium2 (Bass/Tile), 8-core data parallel.

Per batch b: Sigma = F Sp F^T + Q; S = H Sigma H^T + R; KG = Sigma H^T S^-1.
Factored: A = H F; C = Sp A^T; P12 = F C + (H Q)^T; S = H P12 + R
(S symmetric: only the 12 entries the Schur inverse reads are built);
X = S^-1 (2x2-block Schur, approx-recip); KG = P12 X.

Layout: fp16 component-planes [chunk, p, comp, g], prepared HOST-side in
kernel() as part of sharding (g = batch-within-lane, stride-1 innermost;
128 SBUF partitions = batch lanes). The device DMAs planes directly, so
every per-batch product term is an elementwise TT with all operands
stride-1 innermost -> DVE 2x_1P fp16 mode (2 el/cycle/lane, the DVE cap
on TRN2). Contraction sums ride the PE via an fp16 identity stationary
accumulating in PSUM (1 col/cycle); ACT evacuates PSUM->SBUF into the
plane order the next stage needs and also carries the inverse's
scale/copy ops. The SPD 4x4 inverse is a Schur complement on S-planes
batched over IPAIR chunks (fp32, reciprocal_approx_fast on DVE).

Measured on 8xTRN2: DVE-bound (~84% busy); products are the floor.
GPSIMD offload of products or the inverse regresses (chip-wide activity
throttle + 4x worse el/cycle) -- keep KG_ASSIGN/KG_INV at defaults.
"""

import os

import numpy as np

P = 128
B = 262144
NCORES = 8
B_CORE = B // NCORES  # 32768

G = int(os.environ.get("KG_G", "16"))
IPAIR = int(os.environ.get("KG_IPAIR", "8"))
ASSIGN = os.environ.get("KG_ASSIGN", "A:v,C:v,FC:v,HQ:v,S:v,KG:v")
INV_ENG = os.environ.get("KG_INV", "v")
NCHUNK = B_CORE // (P * G)

_NC_CACHE = {}


def _build_nc(g=None, ipair=None, assign=None):
    import concourse.bacc as bacc
    import concourse.mybir as mybir
    import concourse.tile as tile
    from concourse.masks import make_identity

    g = G if g is None else g
    ipair = IPAIR if ipair is None else ipair
    assign = ASSIGN if assign is None else assign

    fp32 = mybir.dt.float32
    fp16 = mybir.dt.float16
    MULT = mybir.AluOpType.mult

    eng_of = dict(kv.split(":") for kv in assign.split(","))

    nchunk = B_CORE // (P * g)
    assert nchunk * P * g == B_CORE
    assert nchunk % ipair == 0
    gi = g * ipair

    nc = bacc.Bacc("TRN2", target_bir_lowering=False, debug=False)

    # Host-prepared fp16 plane-layout inputs: [chunk, p, comp..., g]
    F_d = nc.dram_tensor("Fpl", [nchunk, P, 8, 8, g], fp16, kind="ExternalInput").ap()
    H_d = nc.dram_tensor("Hpl", [nchunk, P, 4, 8, g], fp16, kind="ExternalInput").ap()
    Sp_d = nc.dram_tensor(
        "Sppl", [nchunk, P, 8, 8, g], fp16, kind="ExternalInput"
    ).ap()
    Q_d = nc.dram_tensor("Qpl", [nchunk, P, 8, 8, g], fp16, kind="ExternalInput").ap()
    R_d = nc.dram_tensor("Rpl", [nchunk, P, 4, 4, g], fp16, kind="ExternalInput").ap()
    KG_d = nc.dram_tensor("KG", [B_CORE, 8, 4], fp32, kind="ExternalOutput").ap()

    KGv = KG_d.rearrange("(c p g) i m -> c p g i m", p=P, g=g)

    with tile.TileContext(nc) as tc:
        with (
            tc.tile_pool(name="consts", bufs=1) as consts,
            tc.tile_pool(name="planes", bufs=int(os.environ.get("KG_PLB", "4"))) as plp,
            tc.tile_pool(name="prodac", bufs=int(os.environ.get("KG_ACB", "3"))) as prodacp,
            tc.tile_pool(name="prodfq", bufs=2) as prodfqp,
            tc.tile_pool(name="prodsk", bufs=int(os.environ.get("KG_SKB", "2"))) as prodskp,
            tc.tile_pool(name="mid", bufs=3) as midp,
            tc.tile_pool(name="p12", bufs=IPAIR + 4) as p12p,
            tc.tile_pool(name="sx", bufs=2) as sxp,
            tc.tile_pool(name="invt", bufs=1) as invp,
            tc.tile_pool(name="out", bufs=3) as outp,
            tc.tile_pool(name="psA", bufs=int(os.environ.get("KG_PSB", "6")), space="PSUM") as psmain,
            tc.tile_pool(name="psB", bufs=int(os.environ.get("KG_PSB2", "2")), space="PSUM") as pssml,
        ):
            ident = consts.tile([P, P], fp32, tag="ident")
            make_identity(nc, ident[:])
            id16_t = consts.tile([P, P], fp16, tag="id16")
            nc.vector.tensor_copy(id16_t[:], ident[:])
            id16 = id16_t[:]

            V = nc.vector
            GP = nc.gpsimd

            def ENG(stage):
                return V if eng_of.get(stage, "v") == "v" else GP

            IV = V if INV_ENG == "v" else GP

            st = [dict() for _ in range(nchunk)]
            inv_st = [dict() for _ in range(nchunk // ipair)]

            def emit_load(c):
                s = st[c]
                Fp = plp.tile([P, 8, 8, g], fp16, tag="Fp", name="Fp")
                Hp = plp.tile([P, 4, 8, g], fp16, tag="Hp", name="Hp")
                Spp = plp.tile([P, 8, 8, g], fp16, tag="Spp", name="Spp")
                Qp = plp.tile([P, 8, 8, g], fp16, tag="Qp", name="Qp")
                R16 = plp.tile([P, 4, 4, g], fp16, tag="R16", name="R16")
                nc.sync.dma_start(out=Fp[:], in_=F_d[c])
                nc.sync.dma_start(out=Hp[:], in_=H_d[c])
                nc.sync.dma_start(out=Spp[:], in_=Sp_d[c])
                nc.sync.dma_start(out=Qp[:], in_=Q_d[c])
                nc.sync.dma_start(out=R16[:], in_=R_d[c])
                s["Fp"] = Fp[:]
                s["Hp"] = Hp[:]
                s["Spp"] = Spp[:]
                s["Qp"] = Qp[:]
                s["R16"] = R16[:]

            def emit_A(c):
                # A(m,cc) = sum_j Hp(m,j) Fp(j,cc); evac -> Atp planes (cc,m)
                s = st[c]
                prodA = prodacp.tile([P, 4, 8, 8, g], fp16, tag="prodA", name="prodA")
                ENG("A").tensor_tensor(
                    prodA[:].rearrange("p m j cc g -> p (m j) cc g"),
                    s["Hp"]
                    .rearrange("p m j g -> p (m j) g")
                    .unsqueeze(2)
                    .broadcast_to([P, 32, 8, g]),
                    s["Fp"]
                    .rearrange("p j cc g -> p (j cc) g")
                    .unsqueeze(1)
                    .broadcast_to([P, 4, 64, g]),
                    op=MULT,
                )
                psA = psmain.tile([P, 4 * 8 * g], fp32, tag="ps", name="psA")
                for j in range(8):
                    nc.tensor.matmul(
                        psA[:],
                        id16,
                        prodA[:, :, j].rearrange("p m cc g -> p m (cc g)"),
                        start=(j == 0),
                        stop=(j == 7),
                    )
                s["Atp"] = midp.tile([P, 8, 4, g], fp16, tag="Atp", name="Atp")
                nc.scalar.copy(
                    s["Atp"][:].rearrange("p cc m g -> p m cc g"),
                    psA[:].rearrange("p (m cc g) -> p m cc g", m=4, g=g),
                )

            def emit_C(c):
                # C(i,m) = sum_cc Spp(i,cc) Atp(cc,m); evac -> Cp planes (i,m)
                s = st[c]
                prodC = prodacp.tile([P, 8, 8, 4, g], fp16, tag="prodC", name="prodC")
                ENG("C").tensor_tensor(
                    prodC[:].rearrange("p i cc m g -> p (i cc) m g"),
                    s["Spp"]
                    .rearrange("p i cc g -> p (i cc) g")
                    .unsqueeze(2)
                    .broadcast_to([P, 64, 4, g]),
                    s["Atp"][:]
                    .rearrange("p cc m g -> p (cc m) g")
                    .unsqueeze(1)
                    .broadcast_to([P, 8, 32, g]),
                    op=MULT,
                )
                psC = psmain.tile([P, 8 * 4 * g], fp32, tag="ps", name="psC")
                for cc in range(8):
                    nc.tensor.matmul(
                        psC[:],
                        id16,
                        prodC[:, :, cc].rearrange("p i m g -> p i (m g)"),
                        start=(cc == 0),
                        stop=(cc == 7),
                    )
                s["Cp"] = midp.tile([P, 8, 4, g], fp16, tag="Cp", name="Cp")
                nc.scalar.copy(
                    s["Cp"][:].rearrange("p i m g -> p (i m) g"),
                    psC[:].rearrange("p (im g) -> p im g", g=g),
                )

            def emit_P12(c):
                # P12(i,m) = sum_n Fp(i,n) Cp(n,m) + sum_cc Hp(m,cc) Qp(cc,i)
                s = st[c]
                prodF = prodfqp.tile([P, 8, 8, 4, g], fp16, tag="prodF", name="prodF")
                ENG("FC").tensor_tensor(
                    prodF[:].rearrange("p i n m g -> p (i n) m g"),
                    s["Fp"]
                    .rearrange("p i n g -> p (i n) g")
                    .unsqueeze(2)
                    .broadcast_to([P, 64, 4, g]),
                    s["Cp"][:]
                    .rearrange("p n m g -> p (n m) g")
                    .unsqueeze(1)
                    .broadcast_to([P, 8, 32, g]),
                    op=MULT,
                )
                prodQ = prodfqp.tile([P, 4, 8, 8, g], fp16, tag="prodQ", name="prodQ")
                ENG("HQ").tensor_tensor(
                    prodQ[:].rearrange("p m cc i g -> p (m cc) i g"),
                    s["Hp"]
                    .rearrange("p m cc g -> p (m cc) g")
                    .unsqueeze(2)
                    .broadcast_to([P, 32, 8, g]),
                    s["Qp"]
                    .rearrange("p cc i g -> p (cc i) g")
                    .unsqueeze(1)
                    .broadcast_to([P, 4, 64, g]),
                    op=MULT,
                )
                psP = psmain.tile([P, 8 * 4 * g], fp32, tag="ps", name="psP")
                for n in range(8):
                    nc.tensor.matmul(
                        psP[:],
                        id16,
                        prodF[:, :, n].rearrange("p i m g -> p i (m g)"),
                        start=(n == 0),
                        stop=False,
                    )
                psP_mi = psP[:].rearrange("p (i m g) -> p m i g", i=8, m=4, g=g)
                for cc in range(8):
                    nc.tensor.matmul(
                        psP_mi,
                        id16,
                        prodQ[:, :, cc].rearrange("p m i g -> p m (i g)"),
                        start=False,
                        stop=(cc == 7),
                    )
                s["P12p"] = p12p.tile([P, 8, 4, g], fp16, tag="P12p", name="P12p")
                nc.scalar.copy(
                    s["P12p"][:].rearrange("p i m g -> p (i m) g"),
                    psP[:].rearrange("p (im g) -> p im g", g=g),
                )

            def emit_S(c):
                # S(m,n) = sum_i Hp(m,i) P12p(i,n) + R.  S is symmetric and
                # the Schur inverse never reads the (0:2, 2:4) block, so only
                # compute rows m=0..1 x cols n=0..1 and rows m=2..3 x n=0..3
                # (12 of 16 entries).
                s = st[c]
                prodS = prodskp.tile([P, 12, 8, g], fp16, tag="prodS", name="prodS")
                # per-m TTs keep every AP at <=3 free dims.
                # q slots: m0:{0,1}, m1:{2,3}, m2:{4..7}, m3:{8..11}
                for m, q0, nn in ((0, 0, 2), (1, 2, 2), (2, 4, 4), (3, 8, 4)):
                    ENG("S").tensor_tensor(
                        prodS[:, q0 : q0 + nn].rearrange("p n i g -> p i n g"),
                        s["Hp"][:, m]
                        .unsqueeze(2)
                        .broadcast_to([P, 8, nn, g]),
                        s["P12p"][:, :, 0:nn],
                        op=MULT,
                    )
                psS = pssml.tile([P, 12 * g], fp32, tag="psS", name="psS")
                for i in range(8):
                    nc.tensor.matmul(
                        psS[:],
                        id16,
                        prodS[:, :, i].rearrange("p q g -> p q (g)"),
                        start=(i == 0),
                        stop=False,
                    )
                # R additions for the same 12 (m,n) slots
                nc.tensor.matmul(
                    psS[:, 0 : 4 * g],
                    id16,
                    s["R16"][:, 0:2, 0:2],
                    start=False,
                    stop=False,
                )
                nc.tensor.matmul(
                    psS[:, 4 * g :],
                    id16,
                    s["R16"][:, 2:4],
                    start=False,
                    stop=True,
                )
                b = c // ipair
                ph = c % ipair
                ib = inv_st[b]
                if "S2" not in ib:
                    # q slots: 0,1,4,5 (A block), 8,9,12,13 (B), 10,11,14,15 (D)
                    ib["S2"] = sxp.tile([P, 16, gi], fp32, tag="S2", name="S2")
                S2v = ib["S2"][:, :, ph * g : (ph + 1) * g]
                nc.scalar.copy(
                    S2v.rearrange("p (m n) g -> p m n g", m=4)[:, 0:2, 0:2],
                    psS[:, 0 : 4 * g].rearrange("p (m n g) -> p m n g", m=2, g=g),
                )
                nc.scalar.copy(
                    S2v.rearrange("p (m n) g -> p m n g", m=4)[:, 2:4],
                    psS[:, 4 * g :].rearrange("p (m n g) -> p m n g", m=2, g=g),
                )

            def emit_inv(b):
                ib = inv_st[b]
                S2 = ib["S2"]
                X2 = sxp.tile([P, 16, gi], fp16, tag="X2", name="X2")
                ib["X2"] = X2
                W4 = invp.tile([P, 4, gi], fp32, tag="W4", name="W4")
                W4b = invp.tile([P, 4, gi], fp32, tag="W4b", name="W4b")
                u2 = invp.tile([P, 2, gi], fp32, tag="u2", name="u2")
                d0 = invp.tile([P, gi], fp32, tag="d0", name="d0")
                Pi = invp.tile([P, 2, 2, gi], fp32, tag="Pi", name="Pi")
                pw = invp.tile([P, 2, 2, 2, gi], fp32, tag="pw", name="pw")
                W2b = invp.tile([P, 2, 2, gi], fp32, tag="W2b", name="W2b")
                Sc = invp.tile([P, 2, 2, gi], fp32, tag="Sc", name="Sc")
                X22 = invp.tile([P, 2, 2, gi], fp32, tag="X22", name="X22")
                X21n = invp.tile([P, 2, 2, gi], fp32, tag="X21n", name="X21n")
                t4 = invp.tile([P, 2, 2, gi], fp32, tag="t4", name="t4")

                Sq = S2[:]
                Sblk = S2[:].rearrange("p (m n) g -> p m n g", m=4)

                IV.tensor_tensor(u2[:], Sq[:, 0:2], Sq[:, 5:3:-1], op=MULT)
                IV.tensor_sub(d0[:], u2[:, 0], u2[:, 1])
                V.reciprocal_approx_fast(out=W4[:, 0], in_=d0[:])
                nc.scalar.mul(
                    W4[:, 1:3], W4[:, 0].unsqueeze(1).broadcast_to([P, 2, gi]), -1.0
                )
                nc.scalar.copy(W4[:, 3], W4[:, 0])
                IV.tensor_tensor(
                    Pi[:],
                    Sblk[:, 1::-1, 1::-1],
                    W4[:].rearrange("p (q r) g -> p q r g", q=2),
                    op=MULT,
                )
                Bblk = Sblk[:, 2:4, 0:2]
                for si in range(2):
                    IV.tensor_tensor(
                        pw[:, si],
                        Bblk[:, :, si]
                        .unsqueeze(2)
                        .broadcast_to([P, 2, 2, gi]),
                        Pi[:, si]
                        .unsqueeze(1)
                        .broadcast_to([P, 2, 2, gi]),
                        op=MULT,
                    )
                IV.tensor_add(W2b[:], pw[:, 0], pw[:, 1])
                for si in range(2):
                    IV.tensor_tensor(
                        pw[:, si],
                        W2b[:, :, si]
                        .unsqueeze(2)
                        .broadcast_to([P, 2, 2, gi]),
                        Bblk[:, :, si]
                        .unsqueeze(1)
                        .broadcast_to([P, 2, 2, gi]),
                        op=MULT,
                    )
                IV.tensor_add(t4[:], pw[:, 0], pw[:, 1])
                IV.tensor_sub(
                    Sc[:],
                    Sblk[:, 2:4, 2:4],
                    t4[:],
                )
                Scq = Sc[:].rearrange("p q r g -> p (q r) g")
                IV.tensor_tensor(u2[:], Scq[:, 0:2], Scq[:, 3:1:-1], op=MULT)
                IV.tensor_sub(d0[:], u2[:, 0], u2[:, 1])
                V.reciprocal_approx_fast(out=W4b[:, 0], in_=d0[:])
                nc.scalar.mul(
                    W4b[:, 1:3], W4b[:, 0].unsqueeze(1).broadcast_to([P, 2, gi]), -1.0
                )
                nc.scalar.copy(W4b[:, 3], W4b[:, 0])
                ib["_cont"] = (X2, W4b, Pi, pw, W2b, Sc, X22, X21n, t4)

            def emit_inv_b(b):
                ib = inv_st[b]
                (X2, W4b, Pi, pw, W2b, Sc, X22, X21n, t4) = ib["_cont"]
                IV.tensor_tensor(
                    X22[:],
                    Sc[:, 1::-1, 1::-1],
                    W4b[:].rearrange("p (q r) g -> p q r g", q=2),
                    op=MULT,
                )
                for si in range(2):
                    IV.tensor_tensor(
                        pw[:, si],
                        X22[:, :, si]
                        .unsqueeze(2)
                        .broadcast_to([P, 2, 2, gi]),
                        W2b[:, si]
                        .unsqueeze(1)
                        .broadcast_to([P, 2, 2, gi]),
                        op=MULT,
                    )
                IV.tensor_add(X21n[:], pw[:, 0], pw[:, 1])
                Xblk = X2[:].rearrange("p (n m) g -> p n m g", n=4)
                nc.scalar.mul(Xblk[:, 2:4, 0:2], X21n[:], -1.0)
                nc.scalar.mul(
                    Xblk[:, 0:2, 2:4].rearrange("p n mp g -> p mp n g"),
                    X21n[:],
                    -1.0,
                )
                nc.scalar.copy(Xblk[:, 2:4, 2:4], X22[:])
                for si in range(2):
                    IV.tensor_tensor(
                        pw[:, si],
                        W2b[:, si]
                        .unsqueeze(2)
                        .broadcast_to([P, 2, 2, gi]),
                        X21n[:, si]
                        .unsqueeze(1)
                        .broadcast_to([P, 2, 2, gi]),
                        op=MULT,
                    )
                IV.tensor_add(t4[:], pw[:, 0], pw[:, 1])
                IV.tensor_add(
                    Xblk[:, 0:2, 0:2],
                    Pi[:],
                    t4[:],
                )

            def emit_KG(c):
                s = st[c]
                b = c // ipair
                ph = c % ipair
                X2 = inv_st[b]["X2"]
                Xh = X2[:, :, ph * g : (ph + 1) * g]
                prodK = prodskp.tile([P, 8, 4, 4, g], fp16, tag="prodK", name="prodK")
                ENG("KG").tensor_tensor(
                    prodK[:].rearrange("p i n m g -> p (i n) m g"),
                    s["P12p"][:]
                    .rearrange("p i n g -> p (i n) g")
                    .unsqueeze(2)
                    .broadcast_to([P, 32, 4, g]),
                    Xh.unsqueeze(1).broadcast_to([P, 8, 16, g]),
                    op=MULT,
                )
                psK = psmain.tile([P, 8 * 4 * g], fp32, tag="ps", name="psK")
                for n in range(4):
                    nc.tensor.matmul(
                        psK[:],
                        id16,
                        prodK[:, :, n].rearrange("p i m g -> p i (m g)"),
                        start=(n == 0),
                        stop=(n == 3),
                    )
                KGo = outp.tile([P, g, 8, 4], fp32, tag="KGo", name="KGo")
                nc.scalar.copy(
                    KGo[:].rearrange("p g i m -> p (i m) g"),
                    psK[:].rearrange("p (im g) -> p im g", g=g),
                )
                nc.sync.dma_start(out=KGv[c], in_=KGo[:])

            # waves: L(t) | A(t-1) | P12(t-2) | S(t-3) | C(t-1) |
            #        inv(group ending at t-3) | KG two per wave
            kskew = 4 + ipair
            for t in range(nchunk + kskew + 1):
                if t < nchunk:
                    emit_load(t)
                if 0 <= t - 1 < nchunk:
                    emit_A(t - 1)
                if 0 <= t - 2 < nchunk:
                    emit_P12(t - 2)
                if 0 <= t - 3 < nchunk:
                    emit_S(t - 3)
                if 0 <= t - 1 < nchunk:
                    emit_C(t - 1)
                last_b = nchunk // ipair - 1
                if 0 <= t - 3 < nchunk and (t - 3) % ipair == ipair - 1:
                    bb = (t - 3) // ipair
                    emit_inv(bb)
                    if bb == last_b:
                        # drain: nothing to smooth, finish the inverse now
                        emit_inv_b(bb)
                if 0 <= t - 4 < nchunk and (t - 4) % ipair == ipair - 1:
                    if (t - 4) // ipair != last_b:
                        emit_inv_b((t - 4) // ipair)
                # two KGs per wave starting one wave after inv_b
                o = t - kskew
                if o >= 0 and (o + 1) // ipair >= nchunk // ipair - 1:
                    o = o + 1  # last group's inverse finishes a wave earlier
                if o >= 0:
                    b2 = o // ipair
                    oo = o - b2 * ipair
                    if 0 <= oo < ipair // 2:
                        for c2 in (
                            b2 * ipair + 2 * oo,
                            b2 * ipair + 2 * oo + 1,
                        ):
                            if 0 <= c2 < nchunk:
                                emit_KG(c2)

    nc.compile()
    return nc


def _get_nc():
    if "nc" not in _NC_CACHE:
        _NC_CACHE["nc"] = _build_nc()
    return _NC_CACHE["nc"]


def _planes(x, comp_shape):
    # [B_CORE, *comp] f32 -> [nchunk, P, *comp, G] fp16 contiguous
    x = x.reshape(NCHUNK, P, G, *comp_shape).astype(np.float16)
    nd = x.ndim
    x = np.transpose(x, (0, 1, *range(3, nd), 2))
    return np.ascontiguousarray(x)


def kernel(F, H, Sigma_previous, Q, R):
    from concourse.bass_utils import run_bass_kernel_spmd

    nc = _get_nc()
    in_maps = [host_shard(F, H, Sigma_previous, Q, R, ci) for ci in range(NCORES)]
    res = run_bass_kernel_spmd(nc, in_maps, core_ids=list(range(NCORES)))
    return np.concatenate([r["KG"] for r in res.results], axis=0)


def host_shard(F, H, Sigma_previous, Q, R, ci):
    sl = slice(ci * B_CORE, (ci + 1) * B_CORE)
    return {
        "Fpl": _planes(np.asarray(F[sl], dtype=np.float32), (8, 8)),
        "Hpl": _planes(np.asarray(H[sl], dtype=np.float32), (4, 8)),
        "Sppl": _planes(np.asarray(Sigma_previous[sl], dtype=np.float32), (8, 8)),
        "Qpl": _planes(np.asarray(Q[sl], dtype=np.float32), (8, 8)),
        "Rpl": _planes(np.asarray(R[sl], dtype=np.float32), (4, 4)),
    }
